# revision 26
# baseline (speedup 1.0000x reference)
"""Bidirectional GRU encoder kernel for Trainium2 (8 NeuronCores).

Strategy
--------
Data-parallel over batch: 64 batches -> 8 cores x 8. Each core runs BOTH GRU
directions (two independent recurrent chains pipeline against each other to
hide per-step latency) over all 512 time steps for its batch slice.

Layouts are gate-major: the recurrent matmul uses Whh^T tiles as the
stationary operand so PSUM partitions = gate rows (128-chunks) and the free
dim = batch. All gate math then runs full-width on 128 partitions.

The input projection gx = x @ Wih^T (the bulk of the FLOPs) is computed in
64-step blocks interleaved with the recurrence, entirely in SBUF. The output
projection out = [ys_f, ys_b] @ Wout^T + bout and the hidden projection run
on-chip after the recurrence. Host only shards/transposes inputs and
concatenates per-core outputs.

Numerics: bf16 matmul inputs with fp32 PSUM accumulation, fp32 gate math on
chip. Measured on the 8 trn2 cores: scale-relative absmax error 7.9e-3
(outputs 4.9e-3, hidden 7.9e-3) vs the fp32 jax reference. Cost-model
timeline of the scheduled program: ~0.85 ms/core.

Toolchain note: the walrus build here encodes at most ONE sync wait per
instruction; `legalize_single_wait` splits Tile's multi-wait instructions
into single-wait same-engine NOP chains (required for compilation).
"""

import os
import numpy as np
import ml_dtypes

import concourse.bass as bass
import concourse.mybir as mybir
import concourse.tile as tile
from concourse.bass_utils import run_bass_kernel_spmd

BF16 = ml_dtypes.bfloat16
F32 = np.float32

N_CORES = 8
W_FULL, BS, IN_DIM, H = 512, 64, 1024, 256
B = BS // N_CORES          # batch per core = 8
G = 3 * H                  # gate rows = 768
MC = G // 128              # gate chunks = 6
KC = H // 128              # hidden k-chunks = 2
KX = IN_DIM // 128         # input k-chunks = 8
TBLK = 64                  # gx block size (steps)

dt = mybir.dt
AF = mybir.ActivationFunctionType
OP = mybir.AluOpType

DIRS = ("f", "b")

# results of the last hardware run (test harness reads exec_time_ns off this)
LAST_RESULTS = None


def legalize_single_wait(nc):
    """The walrus build in this environment only encodes ONE sync-wait per
    instruction ("Too many sync wait commands" otherwise). Split each
    multi-wait instruction into single-wait NOPs on the same engine queue
    followed by the instruction carrying the last wait — semantically
    identical on an in-order queue."""
    n_split = 0
    for fn in nc.m.functions:
        for bb in fn.blocks:
            insts = bb.instructions
            new = []
            for inst in insts:
                si = inst.sync_info
                if si is not None and len(si.on_wait) > 1:
                    waits = list(si.on_wait)
                    for k, w in enumerate(waits[:-1]):
                        nop = mybir.InstNoOp(name=f"{inst.name}-nw{k}", ins=[], outs=[])
                        nop.engine = inst.engine
                        nop.sync_info = mybir.SyncInfo(on_wait=[w], on_update=[])
                        new.append(nop)
                    inst.sync_info = mybir.SyncInfo(on_wait=[waits[-1]],
                                                    on_update=list(si.on_update))
                    n_split += 1
                new.append(inst)
            insts[:] = new
    return n_split


def build_nc(W=W_FULL, legalize=True):
    """Build the SPMD Bass program (identical on all cores)."""
    TB = W * B
    assert TB % 128 == 0
    MTB = TB // 128            # out-projection row chunks
    TBLK = min(globals()["TBLK"], W)

    nc = bass.Bass("TRN2", target_bir_lowering=False, debug=False,
                   num_devices=N_CORES)

    xT = nc.dram_tensor("xT", (IN_DIM, TB), dt.bfloat16, kind="ExternalInput").ap()
    wihT = {d: nc.dram_tensor(f"wihT_{d}", (IN_DIM, G), dt.bfloat16,
                              kind="ExternalInput").ap() for d in DIRS}
    whhT = {d: nc.dram_tensor(f"whhT_{d}", (H, G), dt.bfloat16,
                              kind="ExternalInput").ap() for d in DIRS}
    gxbias = {d: nc.dram_tensor(f"gxbias_{d}", (128, MC), dt.float32,
                                kind="ExternalInput").ap() for d in DIRS}
    bhhn = {d: nc.dram_tensor(f"bhhn_{d}", (128, KC * B), dt.float32,
                              kind="ExternalInput").ap() for d in DIRS}
    woutT = nc.dram_tensor("woutT", (2 * H, H), dt.bfloat16, kind="ExternalInput").ap()
    whidT = nc.dram_tensor("whidT", (2 * H, H), dt.bfloat16, kind="ExternalInput").ap()
    bout_bc = nc.dram_tensor("bout_bc", (128, H), dt.float32, kind="ExternalInput").ap()
    bhid_bc = nc.dram_tensor("bhid_bc", (128, H), dt.float32, kind="ExternalInput").ap()

    out_d = nc.dram_tensor("out", (TB, H), dt.float32, kind="ExternalOutput").ap()
    hid_d = nc.dram_tensor("hid", (B, H), dt.float32, kind="ExternalOutput").ap()

    with tile.TileContext(nc) as tc:
        from contextlib import ExitStack
        with ExitStack() as ctx:
            const = ctx.enter_context(tc.tile_pool(name="const", bufs=1))

            # ---- resident weights/biases ----
            whh_sb = {d: [const.tile([128, G], dt.bfloat16, name=f"whh_{d}{k}")
                          for k in range(KC)] for d in DIRS}
            for d in DIRS:
                for k in range(KC):
                    nc.sync.dma_start(out=whh_sb[d][k][:],
                                      in_=whhT[d][k * 128:(k + 1) * 128, :])
            wih_sb = {d: [const.tile([128, G], dt.bfloat16, name=f"wih_{d}{k}")
                          for k in range(KX)] for d in DIRS}
            for d in DIRS:
                for k in range(KX):
                    nc.sync.dma_start(out=wih_sb[d][k][:],
                                      in_=wihT[d][k * 128:(k + 1) * 128, :])
            wout_sb = [const.tile([128, H], dt.bfloat16, name=f"wout{k}")
                       for k in range(4)]
            whid_sb = [const.tile([128, H], dt.bfloat16, name=f"whid{k}")
                       for k in range(4)]
            for k in range(4):
                nc.sync.dma_start(out=wout_sb[k][:], in_=woutT[k * 128:(k + 1) * 128, :])
                nc.sync.dma_start(out=whid_sb[k][:], in_=whidT[k * 128:(k + 1) * 128, :])
            gxb_sb = {d: const.tile([128, MC], dt.float32, name=f"gxb_{d}") for d in DIRS}
            # bhh_n broadcast over batch: [128, KC, B]
            bhhn_sb = {d: const.tile([128, KC * B], dt.float32, name=f"bhhn_{d}")
                       for d in DIRS}
            for d in DIRS:
                nc.sync.dma_start(out=gxb_sb[d][:], in_=gxbias[d][:])
                nc.sync.dma_start(out=bhhn_sb[d][:], in_=bhhn[d][:])
            bout_sb = const.tile([128, H], dt.float32, name="bout_sb")
            bhid_sb = const.tile([128, H], dt.float32, name="bhid_sb")
            nc.sync.dma_start(out=bout_sb[:], in_=bout_bc[:])
            nc.sync.dma_start(out=bhid_sb[:], in_=bhid_bc[:])

            zero2 = const.tile([128, KC, B], dt.bfloat16, name="zero2")
            nc.vector.memset(zero2[:], 0.0)

            # ---- persistent hidden-state history (doubles as GRU outputs) ----
            # ys[d][p, c, t, b] = h_t[b, c*128+p]; bwd chain step s writes t=W-1-s.
            ys = {d: const.tile([128, KC, W, B], dt.bfloat16, name=f"ys_{d}")
                  for d in DIRS}

            # Projection-phase pools are allocated BEFORE the loop pools so
            # their addresses never overlap released loop-pool zones (which
            # would fan a >8-sem wait onto their first instructions).
            osb = ctx.enter_context(tc.tile_pool(name="osb", bufs=3))
            # gx + projection PSUM pool (outer scope: projections reuse the
            # slots after the loop pools release).
            gxps = ctx.enter_context(
                tc.tile_pool(name="gxps", bufs=2, space="PSUM"))

            with ExitStack() as loop_ctx:
                xpool = loop_ctx.enter_context(tc.tile_pool(name="xp", bufs=2))
                gxpool = loop_ctx.enter_context(tc.tile_pool(name="gxp", bufs=2))
                scr = loop_ctx.enter_context(tc.tile_pool(name="scr", bufs=4))
                pspool = loop_ctx.enter_context(
                    tc.tile_pool(name="ps", bufs=3, space="PSUM"))

                # gx blocks are produced by generators whose work items (x
                # DMAs, matmuls, evicts) are SPREAD across the steps of the
                # preceding block, so the PE FIFO never sees a burst that
                # stalls the recurrence.
                gx_tiles = {}        # (d, blk) -> (tile, t0)
                cur_gx = {}
                cur_gx_t0 = {}
                NBLK = W // TBLK

                def gx_work(d, blk):
                    """Yield one emission step at a time for block blk of dir d."""
                    s0 = blk * TBLK
                    t0 = s0 if d == "f" else W - s0 - TBLK
                    xk = [xpool.tile([128, TBLK * B], dt.bfloat16,
                                     name=f"x_{d}{k}_{s0}", tag=f"x{d}{k}")
                          for k in range(KX)]
                    for k in range(KX):
                        nc.sync.dma_start(
                            out=xk[k][:],
                            in_=xT[k * 128:(k + 1) * 128, t0 * B:(t0 + TBLK) * B])
                        yield
                    gxt = gxpool.tile([128, MC, TBLK * B], dt.bfloat16,
                                      name=f"gx_{d}_{s0}", tag=f"gx{d}")
                    gx_tiles[(d, blk)] = (gxt, t0)
                    for m in range(MC):
                        gp = gxps.tile([128, TBLK * B], dt.float32,
                                       name=f"gps_{d}{m}_{s0}", tag="gps")
                        for k in range(KX):
                            nc.tensor.matmul(gp[:], wih_sb[d][k][:, m * 128:(m + 1) * 128],
                                             xk[k][:], start=(k == 0), stop=(k == KX - 1))
                            yield
                        # evict with per-partition bias (bih [+ bhh for r,z])
                        nc.scalar.activation(gxt[:, m, :], gp[:], AF.Identity,
                                             bias=gxb_sb[d][:, m:m + 1], scale=1.0)
                        yield

                def drain(gen):
                    for _ in gen:
                        pass

                # prologue: block 0 of both dirs fully, so step 0 can start
                for d in DIRS:
                    drain(gx_work(d, 0))
                pending = {d: (gx_work(d, 1) if NBLK > 1 else None) for d in DIRS}
                # pull rate: one block (62 items) spread over TBLK steps
                PULL = -(-62 // TBLK) + 1

                def emit_mms(d, s):
                    pst = pspool.tile([128, MC, B], dt.float32,
                                      name=f"ps_{d}_{s}", tag=f"ps{d}")
                    tprev = (s - 1) if d == "f" else (W - s)
                    for k in range(KC):
                        rhs = zero2[:, k, :] if s == 0 else ys[d][:, k, tprev, :]
                        for m in range(MC):
                            nc.tensor.matmul(
                                pst[:, m, :],
                                whh_sb[d][k][:, m * 128:(m + 1) * 128], rhs,
                                start=(k == 0 and m == 0),
                                stop=(k == KC - 1 and m == MC - 1))
                    return pst

                def emit_gates(d, s, pst):
                    tnew = s if d == "f" else W - 1 - s
                    tprev = (s - 1) if d == "f" else (W - s)
                    hprev = zero2[:, :, :] if s == 0 else ys[d][:, :, tprev, :]
                    gxt, t0 = gx_tiles[(d, s // TBLK)]
                    xo = (tnew - t0) * B
                    srz = scr.tile([128, 4, B], dt.float32, name=f"srz_{d}{s}",
                                   tag=f"srz{d}")
                    nc.vector.tensor_add(srz[:], pst[:, 0:4, :],
                                         gxt[:, 0:4, xo:xo + B])
                    sig = scr.tile([128, 4, B], dt.float32, name=f"sig_{d}{s}",
                                   tag=f"sig{d}")
                    nc.scalar.activation(sig[:], srz[:], AF.Sigmoid)
                    uu = scr.tile([128, KC, B], dt.float32, name=f"u_{d}{s}",
                                  tag=f"u{d}")
                    nc.vector.tensor_add(uu[:], pst[:, 4:6, :],
                                         bhhn_sb[d][:].rearrange(
                                             "p (c b) -> p c b", c=KC))
                    vv = scr.tile([128, KC, B], dt.float32, name=f"v_{d}{s}",
                                  tag=f"v{d}")
                    nc.vector.tensor_mul(vv[:], sig[:, 0:2, :], uu[:])
                    t2 = scr.tile([128, KC, B], dt.float32, name=f"t2_{d}{s}",
                                  tag=f"t2{d}")
                    nc.vector.tensor_add(t2[:], vv[:], gxt[:, 4:6, xo:xo + B])
                    nt = scr.tile([128, KC, B], dt.bfloat16, name=f"n_{d}{s}",
                                  tag=f"n{d}")
                    nc.scalar.activation(nt[:], t2[:], AF.Tanh)
                    dd = scr.tile([128, KC, B], dt.bfloat16, name=f"d_{d}{s}",
                                  tag=f"d{d}")
                    nc.gpsimd.tensor_sub(dd[:], hprev, nt[:])
                    ee = scr.tile([128, KC, B], dt.bfloat16, name=f"e_{d}{s}",
                                  tag=f"e{d}")
                    nc.gpsimd.tensor_mul(ee[:], sig[:, 2:4, :], dd[:])
                    nc.vector.tensor_add(ys[d][:, :, tnew, :], nt[:], ee[:])

                # Two chains software-pipelined at a half-iteration offset:
                # iteration s emits [f-MM(s) | b-gates(s-1) | b-MM(s) |
                # f-gates(s)] so each engine FIFO alternates between chains
                # that are half a step apart — one chain's serial latency
                # hides under the other's engine work.
                ps_b_prev = None
                for s in range(W):
                    if s % TBLK == 0:
                        blk = s // TBLK
                        for d in DIRS:
                            if blk > 0 and pending[d] is not None:
                                drain(pending[d])  # leftovers of block blk
                                pending[d] = None
                        if blk + 1 < NBLK:
                            for d in DIRS:
                                pending[d] = gx_work(d, blk + 1)
                    for d in DIRS:
                        g = pending[d]
                        if g is not None:
                            for _ in range(PULL):
                                if next(g, "done") == "done":
                                    pending[d] = None
                                    break
                    ps_f = emit_mms("f", s)
                    if ps_b_prev is not None:
                        emit_gates("b", s - 1, ps_b_prev)
                    ps_b_prev = emit_mms("b", s)
                    emit_gates("f", s, ps_f)
                emit_gates("b", W - 1, ps_b_prev)

            # ---- output projection: out = [ys_f; ys_b] @ Wout^T + bout ----
            if True:
                lhs_list = [("f", 0), ("f", 1), ("b", 0), ("b", 1)]
                for m in range(MTB):
                    po = gxps.tile([128, H], dt.float32, name=f"po_{m}", tag="gps")
                    for kc, (d, c) in enumerate(lhs_list):
                        nc.tensor.matmul(po[:],
                                         ys[d][:, c, m * 16:(m + 1) * 16, :],
                                         wout_sb[kc][:],
                                         start=(kc == 0), stop=(kc == 3))
                    ot = osb.tile([128, H], dt.float32, name=f"ot_{m}", tag="ot")
                    nc.vector.scalar_tensor_tensor(ot[:], po[:], 0.0, bout_sb[:],
                                                   op0=OP.add, op1=OP.add)
                    nc.sync.dma_start(out=out_d[m * 128:(m + 1) * 128, :], in_=ot[:])

                # ---- hidden projection: tanh([h_f; h_b] @ Whid^T + bhid) ----
                ph = gxps.tile([B, H], dt.float32, name="ph", tag="gps")
                for kc, (d, c) in enumerate(lhs_list):
                    tlast = (W - 1) if d == "f" else 0
                    nc.tensor.matmul(ph[:], ys[d][:, c, tlast, :], whid_sb[kc][:],
                                     start=(kc == 0), stop=(kc == 3))
                hf = osb.tile([B, H], dt.float32, name="hf", tag="hf")
                nc.vector.scalar_tensor_tensor(hf[:], ph[:], 0.0, bhid_sb[0:B, :],
                                               op0=OP.add, op1=OP.add)
                ht = osb.tile([B, H], dt.float32, name="ht", tag="ht")
                nc.scalar.activation(ht[:], hf[:], AF.Tanh)
                nc.sync.dma_start(out=hid_d[:], in_=ht[:])

    if legalize:
        legalize_single_wait(nc)
    return nc


def make_inputs(src, Wih_f, Whh_f, bih_f, bhh_f, Wih_b, Whh_b, bih_b, bhh_b,
                Wout, bout, Whid, bhid):
    """Host-side prep: shard batch, transpose to on-chip layouts, cast bf16."""
    W = src.shape[0]
    params = {"f": (Wih_f, Whh_f, bih_f, bhh_f), "b": (Wih_b, Whh_b, bih_b, bhh_b)}
    shared = {}
    for d, (Wih, Whh, bih, bhh) in params.items():
        shared[f"wihT_{d}"] = np.ascontiguousarray(Wih.T).astype(BF16)
        shared[f"whhT_{d}"] = np.ascontiguousarray(Whh.T).astype(BF16)
        gxb = np.empty((128, MC), F32)
        for m in range(MC):
            v = bih[m * 128:(m + 1) * 128].astype(F32)
            if m < 4:  # r,z chunks: fold bhh too
                v = v + bhh[m * 128:(m + 1) * 128]
            gxb[:, m] = v
        shared[f"gxbias_{d}"] = gxb
        bn = np.empty((128, KC, B), F32)
        for j in range(KC):
            bn[:, j, :] = bhh[2 * H + j * 128: 2 * H + (j + 1) * 128, None]
        shared[f"bhhn_{d}"] = bn.reshape(128, KC * B)
    shared["woutT"] = np.ascontiguousarray(Wout.T).astype(BF16)
    shared["whidT"] = np.ascontiguousarray(Whid.T).astype(BF16)
    shared["bout_bc"] = np.tile(bout.astype(F32)[None, :], (128, 1))
    shared["bhid_bc"] = np.tile(bhid.astype(F32)[None, :], (128, 1))

    in_maps = []
    for c in range(N_CORES):
        xs = src[:, c * B:(c + 1) * B, :]                    # (W, B, IN_DIM)
        xTc = np.ascontiguousarray(xs.transpose(2, 0, 1)).reshape(IN_DIM, W * B)
        m = dict(shared)
        m["xT"] = xTc.astype(BF16)
        in_maps.append(m)
    return in_maps


_NC_CACHE = {}


def kernel(**inputs):
    global LAST_RESULTS
    src = np.asarray(inputs["src"], F32)
    W = src.shape[0]
    if W not in _NC_CACHE:
        _NC_CACHE[W] = build_nc(W)
    nc = _NC_CACHE[W]
    in_maps = make_inputs(**{k: np.asarray(v) for k, v in inputs.items()})
    # trace=True needs antenv.axon_hooks, absent in this container; never set it.
    res = run_bass_kernel_spmd(nc, in_maps, list(range(N_CORES)), trace=False)
    LAST_RESULTS = res
    out = np.empty((W, BS, H), F32)
    hid = np.empty((1, BS, H), F32)
    for c in range(N_CORES):
        out[:, c * B:(c + 1) * B, :] = res.results[c]["out"].reshape(W, B, H)
        hid[0, c * B:(c + 1) * B, :] = res.results[c]["hid"]
    return out, hid


# revision 32
# speedup vs baseline: 1.0342x; 1.0342x over previous
"""Bidirectional GRU encoder kernel for Trainium2 (8 NeuronCores).

Strategy
--------
Data-parallel over batch: 64 batches -> 8 cores x 8. Each core runs BOTH GRU
directions (two independent recurrent chains pipeline against each other to
hide per-step latency) over all 512 time steps for its batch slice.

Layouts are gate-major: the recurrent matmul uses Whh^T tiles as the
stationary operand so PSUM partitions = gate rows (128-chunks) and the free
dim = batch. All gate math then runs full-width on 128 partitions.

The input projection gx = x @ Wih^T (the bulk of the FLOPs) is computed in
64-step blocks interleaved with the recurrence, entirely in SBUF. The output
projection out = [ys_f, ys_b] @ Wout^T + bout and the hidden projection run
on-chip after the recurrence. Host only shards/transposes inputs and
concatenates per-core outputs.

Numerics: bf16 matmul inputs with fp32 PSUM accumulation, fp32 gate math on
chip. Measured on the 8 trn2 cores: scale-relative absmax error 7.9e-3
(outputs 4.9e-3, hidden 7.9e-3) vs the fp32 jax reference. Cost-model
timeline of the scheduled program: ~0.85 ms/core.

Toolchain note: the walrus build here encodes at most ONE sync wait per
instruction; `legalize_single_wait` splits Tile's multi-wait instructions
into single-wait same-engine NOP chains (required for compilation).
"""

import os
import numpy as np
import ml_dtypes

import concourse.bass as bass
import concourse.mybir as mybir
import concourse.tile as tile
from concourse.bass_utils import run_bass_kernel_spmd

BF16 = ml_dtypes.bfloat16
F32 = np.float32

N_CORES = 8
W_FULL, BS, IN_DIM, H = 512, 64, 1024, 256
B = BS // N_CORES          # batch per core = 8
G = 3 * H                  # gate rows = 768
MC = G // 128              # gate chunks = 6
KC = H // 128              # hidden k-chunks = 2
KX = IN_DIM // 128         # input k-chunks = 8
TBLK = 64                  # gx block size (steps)

dt = mybir.dt
AF = mybir.ActivationFunctionType
OP = mybir.AluOpType

DIRS = ("f", "b")

# results of the last hardware run (test harness reads exec_time_ns off this)
LAST_RESULTS = None


def legalize_single_wait(nc):
    """The walrus build in this environment only encodes ONE sync-wait per
    instruction ("Too many sync wait commands" otherwise). Split each
    multi-wait instruction into single-wait NOPs on the same engine queue
    followed by the instruction carrying the last wait — semantically
    identical on an in-order queue."""
    n_split = 0
    for fn in nc.m.functions:
        for bb in fn.blocks:
            insts = bb.instructions
            new = []
            for inst in insts:
                si = inst.sync_info
                if si is not None and len(si.on_wait) > 1:
                    waits = list(si.on_wait)
                    for k, w in enumerate(waits[:-1]):
                        nop = mybir.InstNoOp(name=f"{inst.name}-nw{k}", ins=[], outs=[])
                        nop.engine = inst.engine
                        nop.sync_info = mybir.SyncInfo(on_wait=[w], on_update=[])
                        new.append(nop)
                    inst.sync_info = mybir.SyncInfo(on_wait=[waits[-1]],
                                                    on_update=list(si.on_update))
                    n_split += 1
                new.append(inst)
            insts[:] = new
    return n_split


def build_nc(W=W_FULL, legalize=True):
    """Build the SPMD Bass program (identical on all cores)."""
    TB = W * B
    assert TB % 128 == 0
    MTB = TB // 128            # out-projection row chunks
    TBLK = min(globals()["TBLK"], W)

    nc = bass.Bass("TRN2", target_bir_lowering=False, debug=False,
                   num_devices=N_CORES)

    xT = nc.dram_tensor("xT", (IN_DIM, TB), dt.bfloat16, kind="ExternalInput").ap()
    wihT = {d: nc.dram_tensor(f"wihT_{d}", (IN_DIM, G), dt.bfloat16,
                              kind="ExternalInput").ap() for d in DIRS}
    whhT = {d: nc.dram_tensor(f"whhT_{d}", (H, G), dt.bfloat16,
                              kind="ExternalInput").ap() for d in DIRS}
    gxbias = {d: nc.dram_tensor(f"gxbias_{d}", (128, MC), dt.float32,
                                kind="ExternalInput").ap() for d in DIRS}
    bhhn = {d: nc.dram_tensor(f"bhhn_{d}", (128, KC * B), dt.float32,
                              kind="ExternalInput").ap() for d in DIRS}
    woutT = nc.dram_tensor("woutT", (2 * H, H), dt.bfloat16, kind="ExternalInput").ap()
    whidT = nc.dram_tensor("whidT", (2 * H, H), dt.bfloat16, kind="ExternalInput").ap()
    bout_bc = nc.dram_tensor("bout_bc", (128, H), dt.float32, kind="ExternalInput").ap()
    bhid_bc = nc.dram_tensor("bhid_bc", (128, H), dt.float32, kind="ExternalInput").ap()

    out_d = nc.dram_tensor("out", (TB, H), dt.float32, kind="ExternalOutput").ap()
    hid_d = nc.dram_tensor("hid", (B, H), dt.float32, kind="ExternalOutput").ap()

    with tile.TileContext(nc) as tc:
        from contextlib import ExitStack
        with ExitStack() as ctx:
            const = ctx.enter_context(tc.tile_pool(name="const", bufs=1))

            # ---- resident weights/biases ----
            whh_sb = {d: [const.tile([128, G], dt.bfloat16, name=f"whh_{d}{k}")
                          for k in range(KC)] for d in DIRS}
            for d in DIRS:
                for k in range(KC):
                    nc.sync.dma_start(out=whh_sb[d][k][:],
                                      in_=whhT[d][k * 128:(k + 1) * 128, :])
            wih_sb = {d: [const.tile([128, G], dt.bfloat16, name=f"wih_{d}{k}")
                          for k in range(KX)] for d in DIRS}
            for d in DIRS:
                for k in range(KX):
                    nc.sync.dma_start(out=wih_sb[d][k][:],
                                      in_=wihT[d][k * 128:(k + 1) * 128, :])
            wout_sb = [const.tile([128, H], dt.bfloat16, name=f"wout{k}")
                       for k in range(4)]
            whid_sb = [const.tile([128, H], dt.bfloat16, name=f"whid{k}")
                       for k in range(4)]
            for k in range(4):
                nc.sync.dma_start(out=wout_sb[k][:], in_=woutT[k * 128:(k + 1) * 128, :])
                nc.sync.dma_start(out=whid_sb[k][:], in_=whidT[k * 128:(k + 1) * 128, :])
            gxb_sb = {d: const.tile([128, MC], dt.float32, name=f"gxb_{d}") for d in DIRS}
            # bhh_n broadcast over batch: [128, KC, B]
            bhhn_sb = {d: const.tile([128, KC * B], dt.float32, name=f"bhhn_{d}")
                       for d in DIRS}
            for d in DIRS:
                nc.sync.dma_start(out=gxb_sb[d][:], in_=gxbias[d][:])
                nc.sync.dma_start(out=bhhn_sb[d][:], in_=bhhn[d][:])
            bout_sb = const.tile([128, H], dt.float32, name="bout_sb")
            bhid_sb = const.tile([128, H], dt.float32, name="bhid_sb")
            nc.sync.dma_start(out=bout_sb[:], in_=bout_bc[:])
            nc.sync.dma_start(out=bhid_sb[:], in_=bhid_bc[:])

            zero2 = const.tile([128, KC, B], dt.bfloat16, name="zero2")
            nc.vector.memset(zero2[:], 0.0)

            # ---- persistent hidden-state history (doubles as GRU outputs) ----
            # ys[d][p, c, t, b] = h_t[b, c*128+p]; bwd chain step s writes t=W-1-s.
            ys = {d: const.tile([128, KC, W, B], dt.bfloat16, name=f"ys_{d}")
                  for d in DIRS}

            # Projection-phase pools are allocated BEFORE the loop pools so
            # their addresses never overlap released loop-pool zones (which
            # would fan a >8-sem wait onto their first instructions).
            osb = ctx.enter_context(tc.tile_pool(name="osb", bufs=3))
            # gx + projection PSUM pool (outer scope: projections reuse the
            # slots after the loop pools release).
            gxps = ctx.enter_context(
                tc.tile_pool(name="gxps", bufs=2, space="PSUM"))

            with ExitStack() as loop_ctx:
                xpool = loop_ctx.enter_context(tc.tile_pool(name="xp", bufs=2))
                gxpool = loop_ctx.enter_context(tc.tile_pool(name="gxp", bufs=2))
                scr = loop_ctx.enter_context(tc.tile_pool(name="scr", bufs=4))
                pspool = loop_ctx.enter_context(
                    tc.tile_pool(name="ps", bufs=3, space="PSUM"))

                # gx blocks are produced by generators whose work items (x
                # DMAs, matmuls, evicts) are SPREAD across the steps of the
                # preceding block, so the PE FIFO never sees a burst that
                # stalls the recurrence.
                gx_tiles = {}        # (d, blk) -> (tile, t0)
                cur_gx = {}
                cur_gx_t0 = {}
                NBLK = W // TBLK

                def gx_work(d, blk):
                    """Yield one emission step at a time for block blk of dir d."""
                    s0 = blk * TBLK
                    t0 = s0 if d == "f" else W - s0 - TBLK
                    xk = [xpool.tile([128, TBLK * B], dt.bfloat16,
                                     name=f"x_{d}{k}_{s0}", tag=f"x{d}{k}")
                          for k in range(KX)]
                    for k in range(KX):
                        nc.sync.dma_start(
                            out=xk[k][:],
                            in_=xT[k * 128:(k + 1) * 128, t0 * B:(t0 + TBLK) * B])
                        yield
                    gxt = gxpool.tile([128, MC, TBLK * B], dt.bfloat16,
                                      name=f"gx_{d}_{s0}", tag=f"gx{d}")
                    gx_tiles[(d, blk)] = (gxt, t0)
                    for m in range(MC):
                        gp = gxps.tile([128, TBLK * B], dt.float32,
                                       name=f"gps_{d}{m}_{s0}", tag="gps")
                        for k in range(KX):
                            nc.tensor.matmul(gp[:], wih_sb[d][k][:, m * 128:(m + 1) * 128],
                                             xk[k][:], start=(k == 0), stop=(k == KX - 1))
                            yield
                        # evict with per-partition bias (bih [+ bhh for r,z])
                        nc.scalar.activation(gxt[:, m, :], gp[:], AF.Identity,
                                             bias=gxb_sb[d][:, m:m + 1], scale=1.0)
                        yield

                def drain(gen):
                    for _ in gen:
                        pass

                # prologue: block 0 of both dirs fully, so step 0 can start
                for d in DIRS:
                    drain(gx_work(d, 0))
                pending = {d: (gx_work(d, 1) if NBLK > 1 else None) for d in DIRS}
                # pull rate: one block (62 items) spread over TBLK steps
                PULL = -(-62 // TBLK) + 1

                def emit_mms(d, s):
                    pst = pspool.tile([128, MC, B], dt.float32,
                                      name=f"ps_{d}_{s}", tag=f"ps{d}")
                    tprev = (s - 1) if d == "f" else (W - s)
                    for k in range(KC):
                        rhs = zero2[:, k, :] if s == 0 else ys[d][:, k, tprev, :]
                        for m in range(MC):
                            nc.tensor.matmul(
                                pst[:, m, :],
                                whh_sb[d][k][:, m * 128:(m + 1) * 128], rhs,
                                start=(k == 0 and m == 0),
                                stop=(k == KC - 1 and m == MC - 1))
                    return pst

                def emit_gates(d, s, pst):
                    tnew = s if d == "f" else W - 1 - s
                    tprev = (s - 1) if d == "f" else (W - s)
                    hprev = zero2[:, :, :] if s == 0 else ys[d][:, :, tprev, :]
                    gxt, t0 = gx_tiles[(d, s // TBLK)]
                    xo = (tnew - t0) * B
                    srz = scr.tile([128, 4, B], dt.float32, name=f"srz_{d}{s}",
                                   tag=f"srz{d}")
                    nc.vector.tensor_add(srz[:], pst[:, 0:4, :],
                                         gxt[:, 0:4, xo:xo + B])
                    sig = scr.tile([128, 4, B], dt.float32, name=f"sig_{d}{s}",
                                   tag=f"sig{d}")
                    nc.scalar.activation(sig[:], srz[:], AF.Sigmoid)
                    uu = scr.tile([128, KC, B], dt.float32, name=f"u_{d}{s}",
                                  tag=f"u{d}")
                    nc.vector.tensor_add(uu[:], pst[:, 4:6, :],
                                         bhhn_sb[d][:].rearrange(
                                             "p (c b) -> p c b", c=KC))
                    vv = scr.tile([128, KC, B], dt.float32, name=f"v_{d}{s}",
                                  tag=f"v{d}")
                    nc.vector.tensor_mul(vv[:], sig[:, 0:2, :], uu[:])
                    t2 = scr.tile([128, KC, B], dt.float32, name=f"t2_{d}{s}",
                                  tag=f"t2{d}")
                    nc.vector.tensor_add(t2[:], vv[:], gxt[:, 4:6, xo:xo + B])
                    nt = scr.tile([128, KC, B], dt.bfloat16, name=f"n_{d}{s}",
                                  tag=f"n{d}")
                    nc.scalar.activation(nt[:], t2[:], AF.Tanh)
                    dd = scr.tile([128, KC, B], dt.bfloat16, name=f"d_{d}{s}",
                                  tag=f"d{d}")
                    nc.gpsimd.tensor_sub(dd[:], hprev, nt[:])
                    ee = scr.tile([128, KC, B], dt.bfloat16, name=f"e_{d}{s}",
                                  tag=f"e{d}")
                    nc.gpsimd.tensor_mul(ee[:], sig[:, 2:4, :], dd[:])
                    nc.gpsimd.tensor_add(ys[d][:, :, tnew, :], nt[:], ee[:])

                # Two chains software-pipelined at a half-iteration offset:
                # iteration s emits [f-MM(s) | b-gates(s-1) | b-MM(s) |
                # f-gates(s)] so each engine FIFO alternates between chains
                # that are half a step apart — one chain's serial latency
                # hides under the other's engine work.
                ps_b_prev = None
                for s in range(W):
                    if s % TBLK == 0:
                        blk = s // TBLK
                        for d in DIRS:
                            if blk > 0 and pending[d] is not None:
                                drain(pending[d])  # leftovers of block blk
                                pending[d] = None
                        if blk + 1 < NBLK:
                            for d in DIRS:
                                pending[d] = gx_work(d, blk + 1)
                    for d in DIRS:
                        g = pending[d]
                        if g is not None:
                            for _ in range(PULL):
                                if next(g, "done") == "done":
                                    pending[d] = None
                                    break
                    ps_f = emit_mms("f", s)
                    if ps_b_prev is not None:
                        emit_gates("b", s - 1, ps_b_prev)
                    ps_b_prev = emit_mms("b", s)
                    emit_gates("f", s, ps_f)
                emit_gates("b", W - 1, ps_b_prev)

            # ---- output projection: out = [ys_f; ys_b] @ Wout^T + bout ----
            if True:
                lhs_list = [("f", 0), ("f", 1), ("b", 0), ("b", 1)]
                for m in range(MTB):
                    po = gxps.tile([128, H], dt.float32, name=f"po_{m}", tag="gps")
                    for kc, (d, c) in enumerate(lhs_list):
                        nc.tensor.matmul(po[:],
                                         ys[d][:, c, m * 16:(m + 1) * 16, :],
                                         wout_sb[kc][:],
                                         start=(kc == 0), stop=(kc == 3))
                    ot = osb.tile([128, H], dt.float32, name=f"ot_{m}", tag="ot")
                    nc.vector.scalar_tensor_tensor(ot[:], po[:], 0.0, bout_sb[:],
                                                   op0=OP.add, op1=OP.add)
                    nc.sync.dma_start(out=out_d[m * 128:(m + 1) * 128, :], in_=ot[:])

                # ---- hidden projection: tanh([h_f; h_b] @ Whid^T + bhid) ----
                ph = gxps.tile([B, H], dt.float32, name="ph", tag="gps")
                for kc, (d, c) in enumerate(lhs_list):
                    tlast = (W - 1) if d == "f" else 0
                    nc.tensor.matmul(ph[:], ys[d][:, c, tlast, :], whid_sb[kc][:],
                                     start=(kc == 0), stop=(kc == 3))
                hf = osb.tile([B, H], dt.float32, name="hf", tag="hf")
                nc.vector.scalar_tensor_tensor(hf[:], ph[:], 0.0, bhid_sb[0:B, :],
                                               op0=OP.add, op1=OP.add)
                ht = osb.tile([B, H], dt.float32, name="ht", tag="ht")
                nc.scalar.activation(ht[:], hf[:], AF.Tanh)
                nc.sync.dma_start(out=hid_d[:], in_=ht[:])

    if legalize:
        legalize_single_wait(nc)
    return nc


def make_inputs(src, Wih_f, Whh_f, bih_f, bhh_f, Wih_b, Whh_b, bih_b, bhh_b,
                Wout, bout, Whid, bhid):
    """Host-side prep: shard batch, transpose to on-chip layouts, cast bf16."""
    W = src.shape[0]
    params = {"f": (Wih_f, Whh_f, bih_f, bhh_f), "b": (Wih_b, Whh_b, bih_b, bhh_b)}
    shared = {}
    for d, (Wih, Whh, bih, bhh) in params.items():
        shared[f"wihT_{d}"] = np.ascontiguousarray(Wih.T).astype(BF16)
        shared[f"whhT_{d}"] = np.ascontiguousarray(Whh.T).astype(BF16)
        gxb = np.empty((128, MC), F32)
        for m in range(MC):
            v = bih[m * 128:(m + 1) * 128].astype(F32)
            if m < 4:  # r,z chunks: fold bhh too
                v = v + bhh[m * 128:(m + 1) * 128]
            gxb[:, m] = v
        shared[f"gxbias_{d}"] = gxb
        bn = np.empty((128, KC, B), F32)
        for j in range(KC):
            bn[:, j, :] = bhh[2 * H + j * 128: 2 * H + (j + 1) * 128, None]
        shared[f"bhhn_{d}"] = bn.reshape(128, KC * B)
    shared["woutT"] = np.ascontiguousarray(Wout.T).astype(BF16)
    shared["whidT"] = np.ascontiguousarray(Whid.T).astype(BF16)
    shared["bout_bc"] = np.tile(bout.astype(F32)[None, :], (128, 1))
    shared["bhid_bc"] = np.tile(bhid.astype(F32)[None, :], (128, 1))

    in_maps = []
    for c in range(N_CORES):
        xs = src[:, c * B:(c + 1) * B, :]                    # (W, B, IN_DIM)
        xTc = np.ascontiguousarray(xs.transpose(2, 0, 1)).reshape(IN_DIM, W * B)
        m = dict(shared)
        m["xT"] = xTc.astype(BF16)
        in_maps.append(m)
    return in_maps


_NC_CACHE = {}


def kernel(**inputs):
    global LAST_RESULTS
    src = np.asarray(inputs["src"], F32)
    W = src.shape[0]
    if W not in _NC_CACHE:
        _NC_CACHE[W] = build_nc(W)
    nc = _NC_CACHE[W]
    in_maps = make_inputs(**{k: np.asarray(v) for k, v in inputs.items()})
    # trace=True needs antenv.axon_hooks, absent in this container; never set it.
    res = run_bass_kernel_spmd(nc, in_maps, list(range(N_CORES)), trace=False)
    LAST_RESULTS = res
    out = np.empty((W, BS, H), F32)
    hid = np.empty((1, BS, H), F32)
    for c in range(N_CORES):
        out[:, c * B:(c + 1) * B, :] = res.results[c]["out"].reshape(W, B, H)
        hid[0, c * B:(c + 1) * B, :] = res.results[c]["hid"]
    return out, hid


# revision 33
# speedup vs baseline: 1.0364x; 1.0022x over previous
"""Bidirectional GRU encoder kernel for Trainium2 (8 NeuronCores).

Strategy
--------
Data-parallel over batch: 64 batches -> 8 cores x 8. Each core runs BOTH GRU
directions (two independent recurrent chains pipeline against each other to
hide per-step latency) over all 512 time steps for its batch slice.

Layouts are gate-major: the recurrent matmul uses Whh^T tiles as the
stationary operand so PSUM partitions = gate rows (128-chunks) and the free
dim = batch. All gate math then runs full-width on 128 partitions.

The input projection gx = x @ Wih^T (the bulk of the FLOPs) is computed in
64-step blocks interleaved with the recurrence, entirely in SBUF. The output
projection out = [ys_f, ys_b] @ Wout^T + bout and the hidden projection run
on-chip after the recurrence. Host only shards/transposes inputs and
concatenates per-core outputs.

Numerics: bf16 matmul inputs with fp32 PSUM accumulation, fp32 gate math on
chip. Measured on the 8 trn2 cores: scale-relative absmax error 7.9e-3
(outputs 4.9e-3, hidden 7.9e-3) vs the fp32 jax reference. Cost-model
timeline of the scheduled program: ~0.85 ms/core.

Toolchain note: the walrus build here encodes at most ONE sync wait per
instruction; `legalize_single_wait` splits Tile's multi-wait instructions
into single-wait same-engine NOP chains (required for compilation).
"""

import os
import numpy as np
import ml_dtypes

import concourse.bass as bass
import concourse.mybir as mybir
import concourse.tile as tile
from concourse.bass_utils import run_bass_kernel_spmd

BF16 = ml_dtypes.bfloat16
F32 = np.float32

N_CORES = 8
W_FULL, BS, IN_DIM, H = 512, 64, 1024, 256
B = BS // N_CORES          # batch per core = 8
G = 3 * H                  # gate rows = 768
MC = G // 128              # gate chunks = 6
KC = H // 128              # hidden k-chunks = 2
KX = IN_DIM // 128         # input k-chunks = 8
TBLK = 64                  # gx block size (steps)

dt = mybir.dt
AF = mybir.ActivationFunctionType
OP = mybir.AluOpType

DIRS = ("f", "b")

# results of the last hardware run (test harness reads exec_time_ns off this)
LAST_RESULTS = None


def legalize_single_wait(nc):
    """The walrus build in this environment only encodes ONE sync-wait per
    instruction ("Too many sync wait commands" otherwise). Split each
    multi-wait instruction into single-wait NOPs on the same engine queue
    followed by the instruction carrying the last wait — semantically
    identical on an in-order queue."""
    n_split = 0
    for fn in nc.m.functions:
        for bb in fn.blocks:
            insts = bb.instructions
            new = []
            for inst in insts:
                si = inst.sync_info
                if si is not None and len(si.on_wait) > 1:
                    waits = list(si.on_wait)
                    for k, w in enumerate(waits[:-1]):
                        nop = mybir.InstNoOp(name=f"{inst.name}-nw{k}", ins=[], outs=[])
                        nop.engine = inst.engine
                        nop.sync_info = mybir.SyncInfo(on_wait=[w], on_update=[])
                        new.append(nop)
                    inst.sync_info = mybir.SyncInfo(on_wait=[waits[-1]],
                                                    on_update=list(si.on_update))
                    n_split += 1
                new.append(inst)
            insts[:] = new
    return n_split


def build_nc(W=W_FULL, legalize=True):
    """Build the SPMD Bass program (identical on all cores)."""
    TB = W * B
    assert TB % 128 == 0
    MTB = TB // 128            # out-projection row chunks
    TBLK = min(globals()["TBLK"], W)

    nc = bass.Bass("TRN2", target_bir_lowering=False, debug=False,
                   num_devices=N_CORES)

    xT = nc.dram_tensor("xT", (IN_DIM, TB), dt.bfloat16, kind="ExternalInput").ap()
    wihT = {d: nc.dram_tensor(f"wihT_{d}", (IN_DIM, G), dt.bfloat16,
                              kind="ExternalInput").ap() for d in DIRS}
    whhT = {d: nc.dram_tensor(f"whhT_{d}", (H, G), dt.bfloat16,
                              kind="ExternalInput").ap() for d in DIRS}
    gxbias = {d: nc.dram_tensor(f"gxbias_{d}", (128, MC), dt.float32,
                                kind="ExternalInput").ap() for d in DIRS}
    bhhn_blk = {d: nc.dram_tensor(f"bhhn_{d}", (KC * 128, TBLK * B), dt.bfloat16,
                                  kind="ExternalInput").ap().rearrange(
                                      "(c p) n -> p c n", c=KC) for d in DIRS}
    woutT = nc.dram_tensor("woutT", (2 * H, H), dt.bfloat16, kind="ExternalInput").ap()
    whidT = nc.dram_tensor("whidT", (2 * H, H), dt.bfloat16, kind="ExternalInput").ap()
    bout_bc = nc.dram_tensor("bout_bc", (128, H), dt.float32, kind="ExternalInput").ap()
    bhid_bc = nc.dram_tensor("bhid_bc", (128, H), dt.float32, kind="ExternalInput").ap()

    out_d = nc.dram_tensor("out", (TB, H), dt.float32, kind="ExternalOutput").ap()
    hid_d = nc.dram_tensor("hid", (B, H), dt.float32, kind="ExternalOutput").ap()

    with tile.TileContext(nc) as tc:
        from contextlib import ExitStack
        with ExitStack() as ctx:
            const = ctx.enter_context(tc.tile_pool(name="const", bufs=1))

            # ---- resident weights/biases ----
            whh_sb = {d: [const.tile([128, G], dt.bfloat16, name=f"whh_{d}{k}")
                          for k in range(KC)] for d in DIRS}
            for d in DIRS:
                for k in range(KC):
                    nc.sync.dma_start(out=whh_sb[d][k][:],
                                      in_=whhT[d][k * 128:(k + 1) * 128, :])
            wih_sb = {d: [const.tile([128, G], dt.bfloat16, name=f"wih_{d}{k}")
                          for k in range(KX)] for d in DIRS}
            for d in DIRS:
                for k in range(KX):
                    nc.sync.dma_start(out=wih_sb[d][k][:],
                                      in_=wihT[d][k * 128:(k + 1) * 128, :])
            wout_sb = [const.tile([128, H], dt.bfloat16, name=f"wout{k}")
                       for k in range(4)]
            whid_sb = [const.tile([128, H], dt.bfloat16, name=f"whid{k}")
                       for k in range(4)]
            for k in range(4):
                nc.sync.dma_start(out=wout_sb[k][:], in_=woutT[k * 128:(k + 1) * 128, :])
                nc.sync.dma_start(out=whid_sb[k][:], in_=whidT[k * 128:(k + 1) * 128, :])
            gxb_sb = {d: const.tile([128, MC], dt.float32, name=f"gxb_{d}") for d in DIRS}
            for d in DIRS:
                nc.sync.dma_start(out=gxb_sb[d][:], in_=gxbias[d][:])
            bout_sb = const.tile([128, H], dt.float32, name="bout_sb")
            bhid_sb = const.tile([128, H], dt.float32, name="bhid_sb")
            nc.sync.dma_start(out=bout_sb[:], in_=bout_bc[:])
            nc.sync.dma_start(out=bhid_sb[:], in_=bhid_bc[:])

            zero2 = const.tile([128, KC, B], dt.bfloat16, name="zero2")
            nc.vector.memset(zero2[:], 0.0)

            # ---- persistent hidden-state history (doubles as GRU outputs) ----
            # ys[d][p, c, t, b] = h_t[b, c*128+p]; bwd chain step s writes t=W-1-s.
            ys = {d: const.tile([128, KC, W, B], dt.bfloat16, name=f"ys_{d}")
                  for d in DIRS}

            # Projection-phase pools are allocated BEFORE the loop pools so
            # their addresses never overlap released loop-pool zones (which
            # would fan a >8-sem wait onto their first instructions).
            osb = ctx.enter_context(tc.tile_pool(name="osb", bufs=3))
            # gx + projection PSUM pool (outer scope: projections reuse the
            # slots after the loop pools release).
            gxps = ctx.enter_context(
                tc.tile_pool(name="gxps", bufs=2, space="PSUM"))

            with ExitStack() as loop_ctx:
                xpool = loop_ctx.enter_context(tc.tile_pool(name="xp", bufs=2))
                gxpool = loop_ctx.enter_context(tc.tile_pool(name="gxp", bufs=2))
                scr = loop_ctx.enter_context(tc.tile_pool(name="scr", bufs=4))
                pspool = loop_ctx.enter_context(
                    tc.tile_pool(name="ps", bufs=3, space="PSUM"))

                # gx blocks are produced by generators whose work items (x
                # DMAs, matmuls, evicts) are SPREAD across the steps of the
                # preceding block, so the PE FIFO never sees a burst that
                # stalls the recurrence.
                gx_tiles = {}        # (d, blk) -> (tile, t0)
                cur_gx = {}
                cur_gx_t0 = {}
                NBLK = W // TBLK

                def gx_work(d, blk):
                    """Yield one emission step at a time for block blk of dir d."""
                    s0 = blk * TBLK
                    t0 = s0 if d == "f" else W - s0 - TBLK
                    xk = [xpool.tile([128, TBLK * B], dt.bfloat16,
                                     name=f"x_{d}{k}_{s0}", tag=f"x{d}{k}")
                          for k in range(KX)]
                    for k in range(KX):
                        nc.sync.dma_start(
                            out=xk[k][:],
                            in_=xT[k * 128:(k + 1) * 128, t0 * B:(t0 + TBLK) * B])
                        yield
                    gxt = gxpool.tile([128, MC + 2, TBLK * B], dt.bfloat16,
                                      name=f"gx_{d}_{s0}", tag=f"gx{d}")
                    gx_tiles[(d, blk)] = (gxt, t0)
                    # constant bhh_n broadcast lands between rz and n regions
                    # so ONE DVE op covers all six gate pre-activations
                    nc.sync.dma_start(out=gxt[:, 4:6, :], in_=bhhn_blk[d][:])
                    yield
                    for m in range(MC):
                        gp = gxps.tile([128, TBLK * B], dt.float32,
                                       name=f"gps_{d}{m}_{s0}", tag="gps")
                        for k in range(KX):
                            nc.tensor.matmul(gp[:], wih_sb[d][k][:, m * 128:(m + 1) * 128],
                                             xk[k][:], start=(k == 0), stop=(k == KX - 1))
                            yield
                        # evict with per-partition bias (bih [+ bhh for r,z]);
                        # n chunks (m=4,5) land at slab regions 6,7
                        mslot = m if m < 4 else m + 2
                        nc.scalar.activation(gxt[:, mslot, :], gp[:], AF.Identity,
                                             bias=gxb_sb[d][:, m:m + 1], scale=1.0)
                        yield

                def drain(gen):
                    for _ in gen:
                        pass

                # prologue: block 0 of both dirs fully, so step 0 can start
                for d in DIRS:
                    drain(gx_work(d, 0))
                pending = {d: (gx_work(d, 1) if NBLK > 1 else None) for d in DIRS}
                # pull rate: one block (62 items) spread over TBLK steps
                PULL = -(-62 // TBLK) + 1

                def emit_mms(d, s):
                    pst = pspool.tile([128, MC, B], dt.float32,
                                      name=f"ps_{d}_{s}", tag=f"ps{d}")
                    tprev = (s - 1) if d == "f" else (W - s)
                    for k in range(KC):
                        rhs = zero2[:, k, :] if s == 0 else ys[d][:, k, tprev, :]
                        for m in range(MC):
                            nc.tensor.matmul(
                                pst[:, m, :],
                                whh_sb[d][k][:, m * 128:(m + 1) * 128], rhs,
                                start=(k == 0 and m == 0),
                                stop=(k == KC - 1 and m == MC - 1))
                    return pst

                def emit_gates(d, s, pst):
                    tnew = s if d == "f" else W - 1 - s
                    tprev = (s - 1) if d == "f" else (W - s)
                    hprev = zero2[:, :, :] if s == 0 else ys[d][:, :, tprev, :]
                    gxt, t0 = gx_tiles[(d, s // TBLK)]
                    xo = (tnew - t0) * B
                    sall = scr.tile([128, 6, B], dt.float32, name=f"sall_{d}{s}",
                                    tag=f"sall{d}")
                    nc.vector.tensor_add(sall[:], pst[:, 0:6, :],
                                         gxt[:, 0:6, xo:xo + B])
                    sig = scr.tile([128, 4, B], dt.float32, name=f"sig_{d}{s}",
                                   tag=f"sig{d}")
                    nc.scalar.activation(sig[:], sall[:, 0:4, :], AF.Sigmoid)
                    vv = scr.tile([128, KC, B], dt.float32, name=f"v_{d}{s}",
                                  tag=f"v{d}")
                    nc.vector.tensor_mul(vv[:], sig[:, 0:2, :], sall[:, 4:6, :])
                    t2 = scr.tile([128, KC, B], dt.float32, name=f"t2_{d}{s}",
                                  tag=f"t2{d}")
                    nc.vector.tensor_add(t2[:], vv[:], gxt[:, 6:8, xo:xo + B])
                    nt = scr.tile([128, KC, B], dt.bfloat16, name=f"n_{d}{s}",
                                  tag=f"n{d}")
                    nc.scalar.activation(nt[:], t2[:], AF.Tanh)
                    dd = scr.tile([128, KC, B], dt.bfloat16, name=f"d_{d}{s}",
                                  tag=f"d{d}")
                    nc.gpsimd.tensor_sub(dd[:], hprev, nt[:])
                    ee = scr.tile([128, KC, B], dt.bfloat16, name=f"e_{d}{s}",
                                  tag=f"e{d}")
                    nc.gpsimd.tensor_mul(ee[:], sig[:, 2:4, :], dd[:])
                    nc.gpsimd.tensor_add(ys[d][:, :, tnew, :], nt[:], ee[:])

                # Two chains software-pipelined at a half-iteration offset:
                # iteration s emits [f-MM(s) | b-gates(s-1) | b-MM(s) |
                # f-gates(s)] so each engine FIFO alternates between chains
                # that are half a step apart — one chain's serial latency
                # hides under the other's engine work.
                ps_b_prev = None
                for s in range(W):
                    if s % TBLK == 0:
                        blk = s // TBLK
                        for d in DIRS:
                            if blk > 0 and pending[d] is not None:
                                drain(pending[d])  # leftovers of block blk
                                pending[d] = None
                        if blk + 1 < NBLK:
                            for d in DIRS:
                                pending[d] = gx_work(d, blk + 1)
                    for d in DIRS:
                        g = pending[d]
                        if g is not None:
                            for _ in range(PULL):
                                if next(g, "done") == "done":
                                    pending[d] = None
                                    break
                    ps_f = emit_mms("f", s)
                    if ps_b_prev is not None:
                        emit_gates("b", s - 1, ps_b_prev)
                    ps_b_prev = emit_mms("b", s)
                    emit_gates("f", s, ps_f)
                emit_gates("b", W - 1, ps_b_prev)

            # ---- output projection: out = [ys_f; ys_b] @ Wout^T + bout ----
            if True:
                lhs_list = [("f", 0), ("f", 1), ("b", 0), ("b", 1)]
                for m in range(MTB):
                    po = gxps.tile([128, H], dt.float32, name=f"po_{m}", tag="gps")
                    for kc, (d, c) in enumerate(lhs_list):
                        nc.tensor.matmul(po[:],
                                         ys[d][:, c, m * 16:(m + 1) * 16, :],
                                         wout_sb[kc][:],
                                         start=(kc == 0), stop=(kc == 3))
                    ot = osb.tile([128, H], dt.float32, name=f"ot_{m}", tag="ot")
                    nc.vector.scalar_tensor_tensor(ot[:], po[:], 0.0, bout_sb[:],
                                                   op0=OP.add, op1=OP.add)
                    nc.sync.dma_start(out=out_d[m * 128:(m + 1) * 128, :], in_=ot[:])

                # ---- hidden projection: tanh([h_f; h_b] @ Whid^T + bhid) ----
                ph = gxps.tile([B, H], dt.float32, name="ph", tag="gps")
                for kc, (d, c) in enumerate(lhs_list):
                    tlast = (W - 1) if d == "f" else 0
                    nc.tensor.matmul(ph[:], ys[d][:, c, tlast, :], whid_sb[kc][:],
                                     start=(kc == 0), stop=(kc == 3))
                hf = osb.tile([B, H], dt.float32, name="hf", tag="hf")
                nc.vector.scalar_tensor_tensor(hf[:], ph[:], 0.0, bhid_sb[0:B, :],
                                               op0=OP.add, op1=OP.add)
                ht = osb.tile([B, H], dt.float32, name="ht", tag="ht")
                nc.scalar.activation(ht[:], hf[:], AF.Tanh)
                nc.sync.dma_start(out=hid_d[:], in_=ht[:])

    if legalize:
        legalize_single_wait(nc)
    return nc


def make_inputs(src, Wih_f, Whh_f, bih_f, bhh_f, Wih_b, Whh_b, bih_b, bhh_b,
                Wout, bout, Whid, bhid):
    """Host-side prep: shard batch, transpose to on-chip layouts, cast bf16."""
    W = src.shape[0]
    params = {"f": (Wih_f, Whh_f, bih_f, bhh_f), "b": (Wih_b, Whh_b, bih_b, bhh_b)}
    shared = {}
    for d, (Wih, Whh, bih, bhh) in params.items():
        shared[f"wihT_{d}"] = np.ascontiguousarray(Wih.T).astype(BF16)
        shared[f"whhT_{d}"] = np.ascontiguousarray(Whh.T).astype(BF16)
        gxb = np.empty((128, MC), F32)
        for m in range(MC):
            v = bih[m * 128:(m + 1) * 128].astype(F32)
            if m < 4:  # r,z chunks: fold bhh too
                v = v + bhh[m * 128:(m + 1) * 128]
            gxb[:, m] = v
        shared[f"gxbias_{d}"] = gxb
        tb = min(TBLK, W)
        bn = np.empty((KC * 128, tb * B), F32)
        for j in range(KC):
            bn[j * 128:(j + 1) * 128, :] = bhh[2 * H + j * 128:
                                               2 * H + (j + 1) * 128, None]
        shared[f"bhhn_{d}"] = bn.astype(BF16)
    shared["woutT"] = np.ascontiguousarray(Wout.T).astype(BF16)
    shared["whidT"] = np.ascontiguousarray(Whid.T).astype(BF16)
    shared["bout_bc"] = np.tile(bout.astype(F32)[None, :], (128, 1))
    shared["bhid_bc"] = np.tile(bhid.astype(F32)[None, :], (128, 1))

    in_maps = []
    for c in range(N_CORES):
        xs = src[:, c * B:(c + 1) * B, :]                    # (W, B, IN_DIM)
        xTc = np.ascontiguousarray(xs.transpose(2, 0, 1)).reshape(IN_DIM, W * B)
        m = dict(shared)
        m["xT"] = xTc.astype(BF16)
        in_maps.append(m)
    return in_maps


_NC_CACHE = {}


def kernel(**inputs):
    global LAST_RESULTS
    src = np.asarray(inputs["src"], F32)
    W = src.shape[0]
    if W not in _NC_CACHE:
        _NC_CACHE[W] = build_nc(W)
    nc = _NC_CACHE[W]
    in_maps = make_inputs(**{k: np.asarray(v) for k, v in inputs.items()})
    # trace=True needs antenv.axon_hooks, absent in this container; never set it.
    res = run_bass_kernel_spmd(nc, in_maps, list(range(N_CORES)), trace=False)
    LAST_RESULTS = res
    out = np.empty((W, BS, H), F32)
    hid = np.empty((1, BS, H), F32)
    for c in range(N_CORES):
        out[:, c * B:(c + 1) * B, :] = res.results[c]["out"].reshape(W, B, H)
        hid[0, c * B:(c + 1) * B, :] = res.results[c]["hid"]
    return out, hid
